# revision 1
# baseline (speedup 1.0000x reference)
"""Trainium2 Bass kernel for an edge-weighted two-layer sparse MLP (QBAF).

Math (identical to the gather/segment_sum reference):
    out = sigmoid(x @ W1 + b1) @ W2 + b2
where W1 [2048, 1024] / W2 [1024, 8] are densified on host from the
sparse edge lists (scatter-add of per-edge weights; duplicate edges
accumulate, exactly like segment_sum).

Sharding: data-parallel over the batch dim — 8 cores x 512 rows each.
Weights/biases are replicated (W1 is 4 MB in bf16).

On-device layout: everything transposed so the contraction dim sits on
the SBUF partition axis:
    hT = sigmoid(sum_k W1_k^T-slices @ xT_k + b1)   [1024, 512] tiles
    outT = sum_m W2_m^T @ hT_m + b2                 [8, 512]
Layer 1 runs in bf16 (inputs) with fp32 PSUM accumulation; layer 2 in
float32r. Loop order is k-outer / m-inner so all 8 PSUM banks
accumulate while the DMAs stream in, keeping the PE continuously busy
from the first tile. xT and W1 are fused row-block-wise into one DRAM
tensor so each k-step is a single large DMA (per-DMA issue on the sync
sequencer is ~0.5-0.8 us and would otherwise pace the whole kernel).
"""

import sys

import numpy as np

if "/opt/trn_rl_repo" not in sys.path:
    sys.path.insert(0, "/opt/trn_rl_repo")

B = 4096
F = 2048
N1 = 1024
NT = 8
NCORES = 8
BSH = B // NCORES  # 512 batch rows per core
P = 128
K1 = F // P  # 16 contraction tiles, layer 1
M1 = N1 // P  # 8 neuron tiles
K2 = N1 // P  # 8 contraction tiles, layer 2
LXW = BSH + N1  # fused row width: [xT | W1]

# Layer-1 matmul input dtype: bf16 halves DMA traffic and runs the PE at
# 1 cycle/row. Flip to False for float32r layer-1 inputs.
L1_BF16 = True

_CACHE = {}


def _build(l1_bf16=L1_BF16):
    """Trace the Bass/Tile program. Returns the Bass object (uncompiled --
    run_bass_kernel_spmd / bass2jax handles BIR lowering + neuronxcc)."""
    import concourse.bass as bass
    import concourse.mybir as mybir
    import concourse.tile as tile

    dt = mybir.dt
    l1_dt = dt.bfloat16 if l1_bf16 else dt.float32r

    nc = bass.Bass()
    lx = nc.declare_dram_parameter("lx", [F, LXW], l1_dt, isOutput=False)
    w2p = nc.declare_dram_parameter("w2p", [P, K2 * NT], dt.float32r, isOutput=False)
    cn = nc.declare_dram_parameter("cn", [P, M1 + 1], dt.float32, isOutput=False)
    outT = nc.declare_dram_parameter("outT", [NT, BSH], dt.float32, isOutput=True)

    with tile.TileContext(nc) as tc:
        with (
            tc.tile_pool(name="consts", bufs=1) as consts,
            tc.tile_pool(name="lxp", bufs=K1) as lxp,
            tc.tile_pool(name="hp", bufs=M1) as hp,
            tc.tile_pool(name="outp", bufs=1) as outp,
            tc.tile_pool(name="ps", bufs=8, space="PSUM") as ps,
        ):
            # First fused input tiles go out before anything else so the PE
            # can start; descriptor writing is ~1us per DMA per sequencer, so
            # alternate between the two HWDGE rings (SP and ACT) to halve the
            # serial issue time. The tiny const loads ride behind them.
            lxts = []
            for k in range(K1):
                t = lxp.tile([P, LXW], l1_dt, tag="lx", name=f"lx{k}")
                eng = nc.sync if k % 2 == 0 else nc.scalar
                eng.dma_start(out=t[:], in_=lx[k * P : (k + 1) * P, :])
                lxts.append(t)
                if k == 1:
                    w2s = consts.tile([P, K2 * NT], dt.float32r, tag="w2", name="w2s")
                    nc.scalar.dma_start(out=w2s[:], in_=w2p[:])
                    cns = consts.tile([P, M1 + 1], dt.float32, tag="cn", name="cns")
                    nc.scalar.dma_start(out=cns[:], in_=cn[:])
                    # ACT pre-observes the cns DMA semaphore here (off the
                    # critical path); hw allows only ONE wait per ACT
                    # instruction, and the first sigmoid already needs the
                    # PE wait.
                    scr = consts.tile([P, 1], dt.float32, tag="scr", name="scr")
                    nc.scalar.activation(
                        scr[:], cns[:, 0:1], mybir.ActivationFunctionType.Copy
                    )

            # Layer 1: 8 PSUM accumulation groups (one per neuron tile),
            # k-outer so group m only waits on fused tiles k<=current.
            accs = [
                ps.tile([P, BSH], dt.float32, tag="acc", name=f"acc{m}")
                for m in range(M1)
            ]

            # HAM warm-up: the PE clock is gated to 1.2 GHz until ~3.4us of
            # sustained activity. The PE is otherwise idle while the first
            # lx tiles stream in (~10us), so burn that window on dummy
            # matmuls over a memset scratch tile. They write acc bank 0 as
            # self-contained start/stop groups; the real k=0 matmul below
            # resets the bank (start=True), so results are never observed.
            wsc = consts.tile([P, BSH], l1_dt, tag="wsc", name="wsc")
            nc.gpsimd.memset(wsc[:], 0.0)
            for i in range(10):
                nc.tensor.matmul(
                    accs[0][:], wsc[:, 0:P], wsc[:], start=True, stop=True
                )

            for k in range(K1):
                for m in range(M1):
                    nc.tensor.matmul(
                        accs[m][:],
                        lxts[k][:, BSH + m * P : BSH + (m + 1) * P],
                        lxts[k][:, 0:BSH],
                        start=(k == 0),
                        stop=(k == K1 - 1),
                    )

            # sigmoid(acc + b1) -> hT tiles, written as float32r so the
            # layer-2 matmul can consume them in full-rate fp32 mode.
            hts = []
            for m in range(M1):
                ht = hp.tile([P, BSH], dt.float32r, tag="h", name=f"h{m}")
                nc.scalar.activation(
                    ht[:],
                    accs[m][:],
                    mybir.ActivationFunctionType.Sigmoid,
                    bias=cns[:, m : m + 1],
                    scale=1.0,
                )
                hts.append(ht)

            # Layer 2: one [8, 512] accumulation group. 9th 'acc' tile in an
            # 8-buf pool -> reuses the bank freed by the first sigmoid.
            acc2 = ps.tile([P, BSH], dt.float32, tag="acc", name="acc2")
            for m in range(M1):
                nc.tensor.matmul(
                    acc2[:NT, :],
                    w2s[:, m * NT : (m + 1) * NT],
                    hts[m][:],
                    start=(m == 0),
                    stop=(m == M1 - 1),
                )

            outs = outp.tile([NT, BSH], dt.float32, tag="out", name="outs")
            nc.scalar.activation(
                outs[:],
                acc2[:NT, :],
                mybir.ActivationFunctionType.Identity,
                bias=cns[0:NT, M1 : M1 + 1],
                scale=1.0,
            )
            # SWDGE (gpsimd) queue: unused so far, so this carries only the
            # ACT data-dep wait -- hw allows a single wait per instruction,
            # and a sync-queue DMA would also need its lane-reuse wait.
            nc.gpsimd.dma_start(out=outT[:], in_=outs[:])

    return nc


def _strip_start_barrier(nc):
    """Drop the start-of-kernel all-engine drain + EVSEM barrier that Tile
    emits in the 'main' block (~1.5-2us). All Tile semaphores start at 0
    (and this kernel's tail clears them again), and every cross-engine
    dependency inside the kernel is already semaphore-guarded, so engines
    may enter the kernel body unsynchronized."""
    for fn in nc.m.functions:
        for bb in fn.blocks:
            if bb.name == "main":
                bb.instructions = [
                    i
                    for i in bb.instructions
                    if type(i).__name__ not in ("InstDrain", "InstEventSemaphore")
                ]


def _legalize_single_wait(nc):
    """This neuronxcc build allows at most ONE sync wait per instruction
    (setupSyncWait: 'Too many sync wait commands'). Tile emits multi-wait
    instructions (notably the kernel-tail Drain, which waits on every
    engine + DMA lane). Split the extras onto same-engine no-ops placed
    immediately before the instruction."""
    import bass_rust

    for fn in nc.m.functions:
        for bb in fn.blocks:
            out, changed = [], False
            for ins in bb.instructions:
                si = ins.sync_info
                waits = list(si.on_wait) if si is not None else []
                if len(waits) > 1:
                    for i, w in enumerate(waits[:-1]):
                        out.append(
                            bass_rust.InstNoOp(
                                name=f"{ins.name}-sw{i}",
                                engine=ins.engine,
                                ins=[],
                                outs=[],
                                sync_info=bass_rust.SyncInfo(
                                    on_wait=[w], on_update=[]
                                ),
                            )
                        )
                    ins.sync_info = bass_rust.SyncInfo(
                        on_wait=[waits[-1]], on_update=list(si.on_update)
                    )
                    changed = True
                out.append(ins)
            if changed:
                bb.instructions = out


def _densify(w, rows_in, cols_out, n_in, n_out):
    dense = np.zeros((n_in, n_out), np.float32)
    np.add.at(dense, (np.asarray(rows_in), np.asarray(cols_out)), np.asarray(w))
    return dense


def _prep_inputs(x, w1, b1, w2, b2, conn1_out, conn1_in, conn2_out, conn2_in, l1_bf16):
    import ml_dtypes

    ldt = ml_dtypes.bfloat16 if l1_bf16 else np.float32
    x = np.asarray(x, dtype=np.float32)
    W1 = _densify(w1, conn1_in, conn1_out, F, N1).astype(ldt)
    W2 = _densify(w2, conn2_in, conn2_out, N1, NT)
    # w2 packed k-major: w2p[p, k*NT + t] = W2[k*P + p, t]
    w2p = np.ascontiguousarray(
        W2.reshape(K2, P, NT).transpose(1, 0, 2).reshape(P, K2 * NT)
    )
    # consts: cols 0..M1-1 = b1 tiles, col M1 = b2 (on partitions 0..NT-1)
    cn = np.zeros((P, M1 + 1), np.float32)
    cn[:, :M1] = np.asarray(b1, np.float32).reshape(M1, P).T
    cn[:NT, M1] = np.asarray(b2, np.float32)
    xl = x.astype(ldt)
    in_maps = []
    for c in range(NCORES):
        lx = np.empty((F, LXW), ldt)
        lx[:, :BSH] = xl[c * BSH : (c + 1) * BSH, :].T
        lx[:, BSH:] = W1
        in_maps.append({"lx": lx, "w2p": w2p, "cn": cn})
    return in_maps


def _run(inputs, l1_bf16=L1_BF16, trace=False, **run_kwargs):
    """Build (cached), run on the 8 NeuronCores, gather. Returns
    (out [4096, 8] float32, BassKernelResults)."""
    from concourse.bass_utils import run_bass_kernel_spmd

    key = ("nc", l1_bf16)
    if key not in _CACHE:
        nc = _build(l1_bf16)
        # HW-only passes: CoreSim can't schedule post-hoc IR edits, but
        # the split waits are semantically identical for the compiler.
        _strip_start_barrier(nc)
        _legalize_single_wait(nc)
        _CACHE[key] = nc
    nc = _CACHE[key]

    in_maps = _prep_inputs(**inputs, l1_bf16=l1_bf16)
    res = run_bass_kernel_spmd(
        nc, in_maps, list(range(NCORES)), trace=trace, **run_kwargs
    )
    out = np.empty((B, NT), np.float32)
    for c in range(NCORES):
        out[c * BSH : (c + 1) * BSH, :] = res.results[c]["outT"].T
    return out, res


def kernel(**inputs):
    out, _ = _run(inputs)
    return out



# revision 2
# speedup vs baseline: 1.2249x; 1.2249x over previous
"""Trainium2 Bass kernel for an edge-weighted two-layer sparse MLP (QBAF).

Math (identical to the gather/segment_sum reference):
    out = sigmoid(x @ W1 + b1) @ W2 + b2
with W1 [2048, 1024] / W2 [1024, 8] densified on host from the sparse
edge lists (scatter-add; duplicate edges accumulate like segment_sum).

Sharding: data-parallel over batch — 8 cores x 512 rows each; weights
replicated.

Precision/layout: the 2048-deep layer-1 contraction is split into 8
pairs of 256 rows. The first FP8_PAIRS pairs run as fp8(e4m3) DoubleRow
matmuls (256 contraction rows per 512-cycle instruction -- 2x the
bf16/fp16 PE rate); the rest run fp16. Both operand sets are pre-scaled
(x*16, W1*256) so everything shares one fp32 PSUM accumulation per
neuron tile; the sigmoid activation descales by 2^-12. Measured
end-to-end rel err ~1.8e-2 vs the fp32 reference (tolerance 2e-2);
FP8_PAIRS trades accuracy for speed.

Schedule: m-outer over the 8 neuron tiles so each PSUM bank completes
early and its sigmoid + layer-2 matmul overlap the remaining layer-1
work (the ACT-engine sigmoid chain is ~5.5us and would otherwise
serialize at the tail). Each m's fp16 pairs trail one m behind its fp8
run so the late-arriving fp16 DMA never stalls the PE. Inputs stream
over both HWDGE rings (SP + ACT) in PE-consumption order; the ACT ring
issues all its descriptors before the sigmoid chain begins.
"""

import sys

import numpy as np

if "/opt/trn_rl_repo" not in sys.path:
    sys.path.insert(0, "/opt/trn_rl_repo")

B = 4096
F = 2048
N1 = 1024
NT = 8
NCORES = 8
BSH = B // NCORES  # 512 batch rows per core
P = 128
M1 = N1 // P  # 8 neuron tiles
PAIRS = F // (2 * P)  # 8 contraction pairs of 256 rows
FP8_PAIRS = 6
FP16_PAIRS = PAIRS - FP8_PAIRS
SX = 16.0  # x pre-scale (power of 2; keeps fp8 out of subnormals)
SW = 256.0  # W1 pre-scale
WARMUP = 10  # narrow PE matmuls to start the clock ramp before data lands

# DRAM m-block order: ring A streams m 0,2,4,6; ring B streams 1,3,5,7.
# Laying even-m blocks first makes each ring's chunks contiguous.
MORDER = [0, 2, 4, 6, 1, 3, 5, 7]
MPOS = {m: i for i, m in enumerate(MORDER)}

_CACHE = {}


def _build():
    import concourse.bass as bass
    import concourse.mybir as mybir
    import concourse.tile as tile

    dt = mybir.dt
    f8 = dt.float8e4
    f16 = dt.float16
    DR = mybir.MatmulPerfMode.DoubleRow
    SIG = mybir.ActivationFunctionType.Sigmoid

    nc = bass.Bass()
    x8 = nc.declare_dram_parameter("x8", [P, FP8_PAIRS * 2 * BSH], f8, isOutput=False)
    x16 = nc.declare_dram_parameter("x16", [P, FP16_PAIRS * 2 * BSH], f16, isOutput=False)
    w8 = nc.declare_dram_parameter("w8", [P, M1 * FP8_PAIRS * 2 * P], f8, isOutput=False)
    w16 = nc.declare_dram_parameter("w16", [P, M1 * FP16_PAIRS * 2 * P], f16, isOutput=False)
    w2c = nc.declare_dram_parameter("w2c", [P, M1 * NT], f16, isOutput=False)
    cn = nc.declare_dram_parameter("cn", [P, M1 + 1], dt.float32, isOutput=False)
    outT = nc.declare_dram_parameter("outT", [NT, BSH], dt.float32, isOutput=True)

    W8C = FP8_PAIRS * 2 * P  # w8 cols per m-block
    W16C = FP16_PAIRS * 2 * P

    with tile.TileContext(nc) as tc:
        with (
            tc.tile_pool(name="consts", bufs=1) as consts,
            tc.tile_pool(name="xp", bufs=1) as xp,
            tc.tile_pool(name="wp", bufs=1) as wp,
            tc.tile_pool(name="hp", bufs=M1) as hp,
            tc.tile_pool(name="outp", bufs=1) as outp,
            tc.tile_pool(name="ps", bufs=8, space="PSUM") as ps,
        ):
            x8t = {}  # pair j -> ([P, 2, BSH] fp8 tile)
            x16t = {}  # pair jj (0..FP16_PAIRS-1) -> [P, 2, BSH] fp16 tile
            w8t = {}  # m -> (tile, index) with AP tile[:, idx, j, :, :]
            w16t = {}

            def x8_dma(eng, j):
                t = xp.tile([P, 2, BSH], f8, tag=f"x8_{j}", name=f"x8_{j}")
                eng.dma_start(out=t[:], in_=x8[:, j * 2 * BSH : (j + 1) * 2 * BSH])
                x8t[j] = t

            def x16_dma(eng, jj):
                t = xp.tile([P, 2, BSH], f16, tag=f"x16_{jj}", name=f"x16_{jj}")
                eng.dma_start(out=t[:], in_=x16[:, jj * 2 * BSH : (jj + 1) * 2 * BSH])
                x16t[jj] = t

            def w8_dma(eng, ms):
                pos = MPOS[ms[0]]
                assert [MPOS[m] for m in ms] == list(range(pos, pos + len(ms)))
                t = wp.tile(
                    [P, len(ms), FP8_PAIRS, 2, P], f8, tag=f"w8_{ms}", name=f"w8_{ms[0]}"
                )
                eng.dma_start(out=t[:], in_=w8[:, pos * W8C : (pos + len(ms)) * W8C])
                for i, m in enumerate(ms):
                    w8t[m] = (t, i)

            def w16_dma(eng, ms):
                pos = MPOS[ms[0]]
                assert [MPOS[m] for m in ms] == list(range(pos, pos + len(ms)))
                t = wp.tile(
                    [P, len(ms), FP16_PAIRS, 2, P], f16, tag=f"w16_{ms}", name=f"w16_{ms[0]}"
                )
                eng.dma_start(out=t[:], in_=w16[:, pos * W16C : (pos + len(ms)) * W16C])
                for i, m in enumerate(ms):
                    w16t[m] = (t, i)

            # --- ring A (sync / SP sequencer): issue order = stream order
            A = nc.sync
            x8_dma(A, 0)
            w8_dma(A, (0,))
            x8_dma(A, 2)
            w8_dma(A, (2,))
            x8_dma(A, 4)
            w16_dma(A, (0,))
            x16_dma(A, 0)
            w8_dma(A, (4, 6))
            w16_dma(A, (4, 6))
            w16_dma(A, (2,))

            # --- ring B (scalar / ACT sequencer)
            Bq = nc.scalar
            cns = consts.tile([P, M1 + 1], dt.float32, tag="cn", name="cns")
            Bq.dma_start(out=cns[:], in_=cn[:])
            w8_dma(Bq, (1,))
            x8_dma(Bq, 1)
            x8_dma(Bq, 3)
            x8_dma(Bq, 5)
            # dummy sigmoid: pulls the ACT sigmoid table load off the
            # critical path (the first real sigmoid otherwise pays ~1.3us).
            scr = consts.tile([P, 1], dt.float32, tag="scr", name="scr")
            bias0 = consts.tile([P, 1], dt.float32, tag="b0", name="bias0")
            nc.gpsimd.memset(bias0[:], 0.0)
            nc.scalar.activation(scr[:], bias0[:], SIG, bias=bias0[:], scale=1.0)
            w2s = consts.tile([P, M1 * NT], f16, tag="w2", name="w2s")
            Bq.dma_start(out=w2s[:], in_=w2c[:])
            x16_dma(Bq, 1)
            w8_dma(Bq, (3,))
            w16_dma(Bq, (1,))
            w8_dma(Bq, (5, 7))
            w16_dma(Bq, (3,))
            w16_dma(Bq, (5, 7))

            # --- PE: warmup (clock ramp) then m-outer layer 1 + layer 2
            accs = [
                ps.tile([P, BSH], dt.float32, tag="acc", name=f"acc{m}")
                for m in range(M1)
            ]
            wsc = consts.tile([P, P], f16, tag="wsc", name="wsc")
            nc.gpsimd.memset(wsc[:], 0.0)
            for _ in range(WARMUP):
                nc.tensor.matmul(
                    accs[0][:, 0:P], wsc[:], wsc[:], start=True, stop=True
                )

            hts = []

            def l1_fp8(m):
                t, i = w8t[m]
                for j in range(FP8_PAIRS):
                    nc.tensor.matmul(
                        accs[m][:],
                        t[:, i, j],
                        x8t[j][:],
                        start=(j == 0),
                        stop=False,
                        perf_mode=DR,
                    )

            def l1_fp16(m):
                t, i = w16t[m]
                for jj in range(FP16_PAIRS):
                    for s in range(2):
                        last = jj == FP16_PAIRS - 1 and s == 1
                        nc.tensor.matmul(
                            accs[m][:],
                            t[:, i, jj, s],
                            x16t[jj][:, s],
                            start=False,
                            stop=last,
                        )

            def sigmoid(m):
                ht = hp.tile([P, BSH], f16, tag="h", name=f"h{m}")
                nc.scalar.activation(
                    ht[:], accs[m][:], SIG, bias=cns[:, m : m + 1], scale=1.0 / (SX * SW)
                )
                hts.append(ht)

            acc2 = ps.tile([P, BSH], dt.float32, tag="acc", name="acc2")

            def l2(m):
                nc.tensor.matmul(
                    acc2[:NT, :],
                    w2s[:, m * NT : (m + 1) * NT],
                    hts[m][:],
                    start=(m == 0),
                    stop=(m == M1 - 1),
                )

            # PE order: fp8 run of m, fp16 finish of m-1 (one-m lag), L2 of
            # m-2; sigmoids run on ACT as each bank stops.
            l1_fp8(0)
            l1_fp8(1)
            l1_fp16(0)
            l1_fp8(2)
            l1_fp16(1)
            l1_fp8(3)
            l1_fp8(4)
            l1_fp8(5)
            l1_fp8(6)
            l1_fp8(7)
            l1_fp16(2)
            l1_fp16(3)
            l1_fp16(4)
            l1_fp16(5)
            l1_fp16(6)
            l1_fp16(7)

            # (sigmoids are emitted on ACT in order; each waits on its bank's
            # stop. L2 matmuls interleave back into the PE queue.)
            for m in range(M1):
                sigmoid(m)
            for m in range(M1):
                l2(m)

            outs = outp.tile([NT, BSH], dt.float32, tag="out", name="outs")
            nc.vector.tensor_scalar_add(outs[:], acc2[:NT, :], cns[0:NT, M1 : M1 + 1])
            nc.gpsimd.dma_start(out=outT[:], in_=outs[:])

    return nc


def _strip_start_barrier(nc):
    """Drop the start-of-kernel all-engine drain + EVSEM barrier that Tile
    emits in the 'main' block (~1.5-2us). All Tile semaphores start at 0
    (and the kernel tail clears them again), and every cross-engine
    dependency inside the kernel is already semaphore-guarded."""
    for fn in nc.m.functions:
        for bb in fn.blocks:
            if bb.name == "main":
                bb.instructions = [
                    i
                    for i in bb.instructions
                    if type(i).__name__ not in ("InstDrain", "InstEventSemaphore")
                ]


def _legalize_single_wait(nc):
    """This neuronxcc build allows at most ONE sync wait per instruction.
    Split extras onto same-engine no-ops placed immediately before."""
    import bass_rust

    for fn in nc.m.functions:
        for bb in fn.blocks:
            out, changed = [], False
            for ins in bb.instructions:
                si = ins.sync_info
                waits = list(si.on_wait) if si is not None else []
                if len(waits) > 1:
                    for i, w in enumerate(waits[:-1]):
                        out.append(
                            bass_rust.InstNoOp(
                                name=f"{ins.name}-sw{i}",
                                engine=ins.engine,
                                ins=[],
                                outs=[],
                                sync_info=bass_rust.SyncInfo(
                                    on_wait=[w], on_update=[]
                                ),
                            )
                        )
                    ins.sync_info = bass_rust.SyncInfo(
                        on_wait=[waits[-1]], on_update=list(si.on_update)
                    )
                    changed = True
                out.append(ins)
            if changed:
                bb.instructions = out


def _densify(w, rows_in, cols_out, n_in, n_out):
    dense = np.zeros((n_in, n_out), np.float32)
    np.add.at(dense, (np.asarray(rows_in), np.asarray(cols_out)), np.asarray(w))
    return dense


def _prep_inputs(x, w1, b1, w2, b2, conn1_out, conn1_in, conn2_out, conn2_in):
    import ml_dtypes

    f8 = ml_dtypes.float8_e4m3fn
    x = np.asarray(x, np.float32)
    W1 = _densify(w1, conn1_in, conn1_out, F, N1) * SW
    W2 = _densify(w2, conn2_in, conn2_out, N1, NT)

    r8 = FP8_PAIRS * 2 * P  # fp8 contraction rows
    # [j, s, p, m, q] -> [p, (m-ordered) m, j, s, q]
    w8v = W1[:r8].reshape(FP8_PAIRS, 2, P, M1, P).transpose(2, 3, 0, 1, 4)
    w8 = np.ascontiguousarray(w8v[:, MORDER]).reshape(P, -1).astype(f8)
    w16v = W1[r8:].reshape(FP16_PAIRS, 2, P, M1, P).transpose(2, 3, 0, 1, 4)
    w16 = np.ascontiguousarray(w16v[:, MORDER]).reshape(P, -1).astype(np.float16)
    w2c = np.ascontiguousarray(
        W2.reshape(M1, P, NT).transpose(1, 0, 2)
    ).reshape(P, M1 * NT).astype(np.float16)
    cn = np.zeros((P, M1 + 1), np.float32)
    cn[:, :M1] = np.asarray(b1, np.float32).reshape(M1, P).T
    cn[:NT, M1] = np.asarray(b2, np.float32)

    in_maps = []
    for c in range(NCORES):
        xs = x[c * BSH : (c + 1) * BSH].T * SX  # [F, BSH]
        x8 = np.ascontiguousarray(
            xs[:r8].reshape(FP8_PAIRS, 2, P, BSH).transpose(2, 0, 1, 3)
        ).reshape(P, -1).astype(f8)
        x16 = np.ascontiguousarray(
            xs[r8:].reshape(FP16_PAIRS, 2, P, BSH).transpose(2, 0, 1, 3)
        ).reshape(P, -1).astype(np.float16)
        in_maps.append(
            {"x8": x8, "x16": x16, "w8": w8, "w16": w16, "w2c": w2c, "cn": cn}
        )
    return in_maps


def _run(inputs, l1_bf16=True, trace=False, **run_kwargs):
    """Build (cached), run on the 8 NeuronCores, gather. Returns
    (out [4096, 8] float32, BassKernelResults)."""
    from concourse.bass_utils import run_bass_kernel_spmd

    if "nc" not in _CACHE:
        nc = _build()
        _strip_start_barrier(nc)
        _legalize_single_wait(nc)
        _CACHE["nc"] = nc
    nc = _CACHE["nc"]

    in_maps = _prep_inputs(**inputs)
    res = run_bass_kernel_spmd(
        nc, in_maps, list(range(NCORES)), trace=trace, **run_kwargs
    )
    out = np.empty((B, NT), np.float32)
    for c in range(NCORES):
        out[c * BSH : (c + 1) * BSH, :] = res.results[c]["outT"].T
    return out, res


def kernel(**inputs):
    out, _ = _run(inputs)
    return out
